# revision 14
# baseline (speedup 1.0000x reference)
"""nn_GAT — 2-layer PyG-style GAT on 8 TRN2 NeuronCores (Bass/Tile).

Self-contained: kernel(**inputs) takes the FULL unsharded inputs
(as produced by setup_inputs) and returns the FULL [65536, 2] output.

Strategy (graph/data parallel, per sharding hint):
 - nodes sharded 8192/core; edges partitioned by dst core, grouped into
   128-dst-node groups, each group's edges split by src < / >= 32768 (so
   int16 dma_gather indices work via a lo/hi split table) and padded to
   128-edge chunks; all cores share one program (chunk counts maxed).
 - dense phase: per-node record [h(128)|a_s(8)] = x @ [W1 | W1@As] in bf16
   (attention folded in by linearity) + local a_d table in f32 + x@Wp.
 - AllGather the bf16 record table; batched dma_gather (4 SWDGE queues,
   single-packet descriptor streams) pulls 272B records by src and 32B
   a_d rows by (local) dst; segment softmax (shift-free — logits are
   tiny) + message aggregation via one-hot matmuls in PSUM. The one-hot
   tiles are precomputed on the host (static graph) and DMA'd in as bf16,
   shared by both layers.
 - layer 2 repeats with a 16B-row f32 record table [h2|a_s2|a_d2]; its
   segment-sum uses the record as the (3-column) stationary operand so the
   one-hot is the moving operand; final division + bias happen on host
   during unshard (2 flops/output element).
"""
import numpy as np
import ml_dtypes

import concourse.bass as bass
import concourse.bacc as bacc
import concourse.mybir as mybir
import concourse.tile as tile
from concourse.masks import make_identity

F32 = mybir.dt.float32
BF16 = mybir.dt.bfloat16
I16 = mybir.dt.int16
NEG_SLOPE = 0.2
BF = ml_dtypes.bfloat16

N_NODES = 65536
IN_FEAT = 768
N_CORES = 8
HEADS = 8
C1 = 16
OUT_FEAT = 2
SUP = 512
HALF = 32768  # int16 split point for gather tables
NQ = 4        # SWDGE queues (Q7 core pairs) to spread gathers over


class _Cfg:
    def __init__(self):
        self.N, self.IN, self.NC = N_NODES, IN_FEAT, N_CORES
        self.HEADS, self.C1, self.OUT, self.SUP = HEADS, C1, OUT_FEAT, SUP
        self.calo = []   # per-group lo-segment chunk counts
        self.cahi = []   # per-group hi-segment chunk counts

    @property
    def NL(self):
        return self.N // self.NC

    @property
    def G(self):
        return self.NL // 128

    @property
    def KT(self):
        return self.IN // 128

    @property
    def HID(self):
        return self.HEADS * self.C1

    @property
    def cg(self):
        return [a + b for a, b in zip(self.calo, self.cahi)]

    @property
    def Tlo(self):
        return sum(self.calo)

    @property
    def Thi(self):
        return sum(self.cahi)

    @property
    def T(self):
        return self.Tlo + self.Thi


def _wrap16(vals_by_slot, ncols):
    """vals_by_slot: int array indexed by slot j -> idx value.
    Returns [128, ncols] int16 with idx j at [j%16, j//16], replicated 8x."""
    a = np.zeros((16, ncols), np.int16)
    n = len(vals_by_slot)
    j = np.arange(n)
    a[j % 16, j // 16] = vals_by_slot
    return np.tile(a, (8, 1))


def _host_prep(cfg, x, edge_index, W1, att_src1, att_dst1, b1, Wp, bp,
               W2, att_src2, att_dst2, b2):
    N, NC, NL, G = cfg.N, cfg.NC, cfg.NL, cfg.G
    HID = cfg.HID
    x = np.asarray(x, np.float32)
    ei = np.asarray(edge_index)
    loops = np.arange(N, dtype=np.int64)
    src = np.concatenate([ei[0], loops]).astype(np.int64)
    dst = np.concatenate([ei[1], loops]).astype(np.int64)
    # sort by (dst, src>=HALF) so each 128-dst group splits into lo/hi runs
    order = np.lexsort((src >= HALF, dst // 128))
    src, dst = src[order], dst[order]

    core_of = dst // NL
    core_starts = np.searchsorted(core_of, np.arange(NC + 1))

    # per-core per-group lo/hi counts
    nlo = np.zeros((NC, G), np.int64)
    nhi = np.zeros((NC, G), np.int64)
    for k in range(NC):
        s, e = core_starts[k], core_starts[k + 1]
        g = (dst[s:e] - k * NL) // 128
        hi = (src[s:e] >= HALF).astype(np.int64)
        nlo[k] = np.bincount(g, weights=1 - hi, minlength=G)
        nhi[k] = np.bincount(g, weights=hi, minlength=G)
    calo = np.maximum((-(-nlo.astype(np.int64) // 128)).max(axis=0), 1)
    cahi = np.maximum((-(-nhi.astype(np.int64) // 128)).max(axis=0), 1)
    cfg.calo = [int(c) for c in calo]
    cfg.cahi = [int(c) for c in cahi]
    cg = calo + cahi
    offlo = np.concatenate([[0], np.cumsum(calo)])
    offhi = np.concatenate([[0], np.cumsum(cahi)])
    off = np.concatenate([[0], np.cumsum(cg)])
    Tlo, Thi, T = int(calo.sum()), int(cahi.sum()), int(cg.sum())

    W1 = np.asarray(W1, np.float32)
    Wp = np.asarray(Wp, np.float32)
    As = np.zeros((HID, cfg.HEADS), np.float32)
    Ad = np.zeros((HID, cfg.HEADS), np.float32)
    hh = np.repeat(np.arange(cfg.HEADS), cfg.C1)
    As[np.arange(HID), hh] = np.asarray(att_src1, np.float32).ravel()
    Ad[np.arange(HID), hh] = np.asarray(att_dst1, np.float32).ravel()
    WBIG = np.concatenate([W1, W1 @ As, W1 @ Ad, Wp], axis=1)  # [IN, 272]

    W2 = np.asarray(W2, np.float32)
    M4 = np.concatenate(
        [W2, W2 @ np.asarray(att_src2, np.float32).T,
         W2 @ np.asarray(att_dst2, np.float32).T], axis=1)     # [HID, 4]
    BC1 = (np.asarray(b1, np.float32) + np.asarray(bp, np.float32))[None, :]
    B2 = np.asarray(b2, np.float32)

    in_maps = []
    for k in range(NC):
        s, e = core_starts[k], core_starts[k + 1]
        sk, dk = src[s:e], dst[s:e]
        gk = (dk - k * NL) // 128
        hik = sk >= HALF
        # rank within (group, seg): edges are sorted by (group, hi) so
        # positions within each (g, seg) run are consecutive
        pos = np.arange(e - s)
        seg_key = gk * 2 + hik
        seg_start = np.concatenate([[0], np.cumsum(np.bincount(
            seg_key, minlength=2 * G))])
        r_in_seg = pos - seg_start[seg_key]
        # chunk slot j within the group's chunk space
        j_lo = r_in_seg                     # for lo edges
        j_hi = calo[gk] * 128 + r_in_seg    # for hi edges
        j = np.where(hik, j_hi, j_lo)
        chunk = off[gk] + j // 128          # global chunk index
        lane = j % 128

        # lo/hi gather slot (position within that segment's idx stream)
        slot_lo = (offlo[gk] * 128 + r_in_seg)[~hik]
        slot_hi = (offhi[gk] * 128 + r_in_seg)[hik]
        vals = np.zeros(Tlo * 128, np.int64)
        vals[slot_lo] = sk[~hik]
        SRCLO = _wrap16(vals, 8 * Tlo)
        vals = np.zeros(Thi * 128, np.int64)
        vals[slot_hi] = sk[hik] - HALF
        SRCHI = _wrap16(vals, 8 * Thi)
        # one-hot tiles (static graph): chunk c col-block holds
        # oh[lane, dstpos] = 1 for each real edge; padding rows all-zero,
        # plus the transposed tiles for dst->edge broadcasts (a_d terms)
        dpos = (dk - k * NL) % 128
        OH = np.zeros((128, T * 128), BF)
        OH[lane, chunk * 128 + dpos] = 1
        OHT = np.zeros((128, T * 128), BF)
        OHT[dpos, chunk * 128 + lane] = 1

        in_maps.append({
            "XT": np.ascontiguousarray(
                x[k * NL:(k + 1) * NL].T).astype(BF),
            "WBIG": WBIG.astype(BF), "M4": M4, "BC1": BC1,
            "SRCLO": SRCLO, "SRCHI": SRCHI, "OH": OH, "OHT": OHT,
        })
    return cfg, in_maps, B2


def _unshard(cfg, outs, B2):
    parts = []
    for k in range(cfg.NC):
        o = outs[k]["OUT"]                       # [4, G*128]
        num = o[0:2].reshape(2, cfg.G, 128)
        den = o[2].reshape(cfg.G, 128)
        r = (num / den[None]).transpose(1, 2, 0).reshape(cfg.NL, 2)
        parts.append(r + B2[None, :])
    return np.concatenate(parts, axis=0).astype(np.float32)


MAX_GCH = 16  # chunks (x128 idxs) per dma_gather call


def _gather(nc, out_tile, out_col0, in_ap, idx_tile, idx_col0, nchunks, elem,
            stride_bytes, queue_num):
    """Batched dma_gather, split into <=MAX_GCH-chunk calls.
    out rows: [128, nchunks, elem] at out_tile cols out_col0*elem;
    idx cols: idx_tile[:, 8*idx_col0 : 8*(idx_col0+nchunks)]."""
    eng = nc.gpsimd
    insts = []
    for c0 in range(0, nchunks, MAX_GCH):
        nch = min(MAX_GCH, nchunks - c0)
        o = out_tile[:, (out_col0 + c0) * elem:(out_col0 + c0 + nch) * elem]
        o = o.rearrange("p (n e) -> p n e", e=elem)
        ix = idx_tile[:, 8 * (idx_col0 + c0):8 * (idx_col0 + c0 + nch)]
        insts.append(eng.add_instruction(
            mybir.InstDMAGatherAnt(
                name=nc.get_next_instruction_name(),
                ins=[*eng.lower_ap_dma(in_ap, for_custom_bir_dma=True),
                     eng.lower_ap(ix),
                     eng.lower_val_access(eng.to_reg(nch * 128))],
                outs=[eng.lower_ap(o)],
                transpose=False,
                num_idxs=nch * 128,
                elem_size=elem,
                stride_bytes_256=stride_bytes // 256,
                gen_mode=0,
                single_packet=False,
                queue_num=queue_num,
            )))
    return insts


def _build(cfg):
    NC = cfg.NC
    NL, G, KT = cfg.NL, cfg.G, cfg.KT
    H, C1_, HID, OUT = cfg.HEADS, cfg.C1, cfg.HID, cfg.OUT
    REC = HID + 2 * H        # 144 (gemm out: h|a_s|a_d)
    GREC = HID + H           # 136 (gathered: h|a_s)
    TROW = 256               # bf16 elements per T1main row (512B)
    ADW = 64                 # f32 per T1ad row (256B)
    R2W = 64                 # f32 per R2main row (256B)
    SUP_ = cfg.SUP
    calo, cahi, cg = cfg.calo, cfg.cahi, cfg.cg
    Tlo, Thi, T = cfg.Tlo, cfg.Thi, cfg.T
    offlo = [0]
    for c in calo:
        offlo.append(offlo[-1] + c)
    offhi = [0]
    for c in cahi:
        offhi.append(offhi[-1] + c)
    off = [0]
    for c in cg:
        off.append(off[-1] + c)

    qrot = [0]

    def nextq():
        q = qrot[0]
        qrot[0] = (q + 1) % NQ
        return q

    nc = bacc.Bacc("TRN2", target_bir_lowering=False, debug=False,
                   num_devices=NC, num_swdge_queues=NQ)
    XT = nc.dram_tensor("XT", [cfg.IN, NL], BF16, kind="ExternalInput")
    WBIGd = nc.dram_tensor("WBIG", [cfg.IN, REC + HID], BF16, kind="ExternalInput")
    M4d = nc.dram_tensor("M4", [HID, 4], F32, kind="ExternalInput")
    BC1d = nc.dram_tensor("BC1", [1, HID], F32, kind="ExternalInput")
    SRCLOd = nc.dram_tensor("SRCLO", [128, 8 * Tlo], I16, kind="ExternalInput")
    SRCHId = nc.dram_tensor("SRCHI", [128, 8 * Thi], I16, kind="ExternalInput")
    OHd = nc.dram_tensor("OH", [128, T * 128], BF16, kind="ExternalInput")
    OHTd = nc.dram_tensor("OHT", [128, T * 128], BF16, kind="ExternalInput")
    OUTd = nc.dram_tensor("OUT", [4, G * 128], F32, kind="ExternalOutput")

    with tile.TileContext(nc) as tc:
        with (
            tc.tile_pool(name="dram", bufs=1, space="DRAM") as dram,
            tc.tile_pool(name="const", bufs=1) as cb,
            tc.tile_pool(name="persist", bufs=1) as pp,
        ):
            T1loc = dram.tile([NL, TROW], BF16)
            T1main = dram.tile([cfg.N, TROW], BF16)
            R2loc = dram.tile([NL, 4], F32)
            R2allc = dram.tile([cfg.N, 4], F32)
            R2main = dram.tile([cfg.N, R2W], F32)

            ident = cb.tile([128, 128], F32)
            make_identity(nc, ident[:])
            wb_sb = []
            for kk in range(KT):
                t = cb.tile([128, REC + HID], BF16, tag=f"wb{kk}", name=f"wb{kk}")
                nc.sync.dma_start(t[:], WBIGd[kk * 128:(kk + 1) * 128, :])
                wb_sb.append(t)
            m4_sb = cb.tile([HID, 4], F32)
            nc.sync.dma_start(m4_sb[:], M4d[:])
            ones1 = cb.tile([1, 128], F32)
            nc.vector.memset(ones1[:], 1.0)
            bc1row = cb.tile([1, HID], F32)
            nc.sync.dma_start(bc1row[:], BC1d[:])
            with tc.tile_pool(name="bpsum", bufs=1, space="PSUM") as bps:
                bp1 = bps.tile([128, HID], F32)
                nc.tensor.matmul(bp1[:], lhsT=ones1[:], rhs=bc1row[:], start=True, stop=True)
                BC1T = cb.tile([128, HID], F32)
                nc.vector.tensor_copy(BC1T[:], bp1[:])

            p_sb = pp.tile([128, G * HID], F32)
            ad_sb = pp.tile([128, G * H], BF16)
            r2stage = pp.tile([128, G * 4], F32)
            outstage = pp.tile([4, G * 128], F32)
            nc.vector.memset(outstage[:], 0.0)

            # ---------------- phase A: GEMM
            n_sup = NL // SUP_
            m_per = SUP_ // 128
            with (
                tc.tile_pool(name="xts", bufs=2 * KT) as xp,
                tc.tile_pool(name="gpsum", bufs=3, space="PSUM") as gps,
                tc.tile_pool(name="grec", bufs=3) as grp,
            ):
                for s in range(n_sup):
                    xts = []
                    for kk in range(KT):
                        t = xp.tile([128, SUP_], BF16, tag="xts", name="xts")
                        nc.sync.dma_start(
                            t[:], XT[kk * 128:(kk + 1) * 128, s * SUP_:(s + 1) * SUP_])
                        xts.append(t)
                    for m in range(m_per):
                        gm = s * m_per + m
                        ps = gps.tile([128, REC + HID], F32, tag="gp", name="gp")
                        for kk in range(KT):
                            nc.tensor.matmul(
                                ps[:], lhsT=xts[kk][:, m * 128:(m + 1) * 128],
                                rhs=wb_sb[kk][:], start=(kk == 0), stop=(kk == KT - 1))
                        rec = grp.tile([128, TROW], BF16, tag="rec", name="rec")
                        nc.vector.tensor_copy(rec[:, 0:GREC], ps[:, 0:GREC])
                        nc.vector.tensor_copy(
                            ad_sb[:, gm * H:(gm + 1) * H], ps[:, GREC:REC])
                        nc.vector.tensor_copy(
                            p_sb[:, gm * HID:(gm + 1) * HID], ps[:, REC:REC + HID])
                        nc.sync.dma_start(T1loc[gm * 128:(gm + 1) * 128, :], rec[:])

            # ---------------- phase B: AllGather T1 (padded bf16 rows)
            cc1 = nc.gpsimd.collective_compute(
                "AllGather", mybir.AluOpType.bypass,
                replica_groups=[list(range(NC))],
                ins=[T1loc.opt()], outs=[T1main.opt()])

            T1lo_h = T1main[:][0:HALF, 0:GREC]
            T1hi_h = T1main[:][HALF:cfg.N, 0:GREC]

            # ---------------- phase C: layer-1 edge pass + layer-2 prep
            GB = 2  # groups per gather batch
            with (
                tc.tile_pool(name="erec", bufs=3) as ep,
                tc.tile_pool(name="ework", bufs=4) as ew,
                tc.tile_pool(name="escall", bufs=2) as esc,
                tc.tile_pool(name="eoh", bufs=2) as eoh,
                tc.tile_pool(name="epsum", bufs=2, space="PSUM") as eps,
                tc.tile_pool(name="apsum", bufs=2, space="PSUM") as aps,
                tc.tile_pool(name="tpsum", bufs=2, space="PSUM") as tps,
            ):
                for g0 in range(0, G, GB):
                    gs = list(range(g0, min(g0 + GB, G)))
                    nblo = offlo[gs[-1] + 1] - offlo[g0]
                    nbhi = offhi[gs[-1] + 1] - offhi[g0]
                    nb = off[gs[-1] + 1] - off[g0]
                    silo = ep.tile([128, 8 * nblo], I16, tag="silo", name="silo")
                    nc.sync.dma_start(
                        silo[:], SRCLOd[:, 8 * offlo[g0]:8 * (offlo[g0] + nblo)])
                    sihi = ep.tile([128, 8 * nbhi], I16, tag="sihi", name="sihi")
                    nc.sync.dma_start(
                        sihi[:], SRCHId[:, 8 * offhi[g0]:8 * (offhi[g0] + nbhi)])
                    oht = eoh.tile([128, nb * 128], BF16, tag="oht", name="oht")
                    nc.sync.dma_start(
                        oht[:], OHd[:, off[g0] * 128:(off[g0] + nb) * 128])
                    ohtT = eoh.tile([128, nb * 128], BF16, tag="ohtT", name="ohtT")
                    nc.sync.dma_start(
                        ohtT[:], OHTd[:, off[g0] * 128:(off[g0] + nb) * 128])
                    hlo = ep.tile([128, nblo * GREC], BF16, tag="hlo", name="hlo")
                    for gi in _gather(nc, hlo[:], 0, T1lo_h, silo[:], 0,
                                      nblo, GREC, TROW * 2, nextq()):
                        tile.add_dep_helper(gi.ins, cc1.ins, sync=True, reason="ag1")
                    hhi = ep.tile([128, nbhi * GREC], BF16, tag="hhi", name="hhi")
                    for gi in _gather(nc, hhi[:], 0, T1hi_h, sihi[:], 0,
                                      nbhi, GREC, TROW * 2, nextq()):
                        tile.add_dep_helper(gi.ins, cc1.ins, sync=True, reason="ag1")
                    for g in gs:
                        # per-edge a_d via one-hot-transpose matmuls (PSUM)
                        gb0 = off[g] - off[g0]
                        adp = aps.tile([128, cg[g] * H], F32, tag="adp", name="adp")
                        for i in range(cg[g]):
                            nc.tensor.matmul(
                                adp[:, i * H:(i + 1) * H],
                                lhsT=ohtT[:, (gb0 + i) * 128:(gb0 + i + 1) * 128],
                                rhs=ad_sb[:, g * H:(g + 1) * H],
                                start=True, stop=True)
                        # segment (tile, local chunk range) resolution
                        segs = [
                            (hlo, offlo[g] - offlo[g0], calo[g], 0),
                            (hhi, offhi[g] - offhi[g0], cahi[g], calo[g]),
                        ]
                        scall = esc.tile([128, cg[g] * GREC], BF16,
                                         tag="scall", name="scall")
                        base = off[g] - off[g0]
                        for (ht, lc0, nseg, cbase) in segs:
                            # group+segment-wide batched ops
                            as_ap = bass.AP(
                                ht.tensor, ht[:].offset + lc0 * GREC + HID,
                                [ht[:].ap[0], [GREC, nseg], [1, H]])
                            ad_ap = bass.AP(
                                adp.tensor,
                                adp[:].offset + cbase * H,
                                [adp[:].ap[0], [H, nseg], [1, H]])
                            epre = ew.tile([128, nseg * H], F32, tag="epre", name="epre")
                            nc.vector.tensor_tensor(
                                out=epre[:].rearrange("p (n h) -> p n h", h=H),
                                in0=as_ap, in1=ad_ap, op=mybir.AluOpType.add)
                            lr = ew.tile([128, nseg * H], F32, tag="lr", name="lr")
                            nc.vector.tensor_scalar_mul(lr[:], epre[:], NEG_SLOPE)
                            lrm = ew.tile([128, nseg * H], F32, tag="lrm", name="lrm")
                            nc.vector.tensor_tensor(
                                out=lrm[:], in0=epre[:], in1=lr[:],
                                op=mybir.AluOpType.max)
                            ex_ap = bass.AP(
                                scall.tensor, scall[:].offset + cbase * GREC + HID,
                                [scall[:].ap[0], [GREC, nseg], [1, H]])
                            nc.scalar.activation(
                                ex_ap, lrm[:].rearrange("p (n h) -> p n h", h=H),
                                mybir.ActivationFunctionType.Exp)
                            # 4-dim scaled-message mul (bf16 h x bf16 ex -> bf16)
                            out4 = bass.AP(
                                scall.tensor, scall[:].offset + cbase * GREC,
                                [scall[:].ap[0], [GREC, nseg], [C1_, H], [1, C1_]])
                            in04 = bass.AP(
                                ht.tensor, ht[:].offset + lc0 * GREC,
                                [ht[:].ap[0], [GREC, nseg], [C1_, H], [1, C1_]])
                            in14 = bass.AP(
                                scall.tensor, scall[:].offset + cbase * GREC + HID,
                                [scall[:].ap[0], [GREC, nseg], [1, H], [0, C1_]])
                            nc.vector.tensor_tensor(
                                out=out4, in0=in04, in1=in14, op=mybir.AluOpType.mult)

                        psg = eps.tile([128, GREC], F32, tag="psg", name="psg")
                        for i in range(cg[g]):
                            ohc = (off[g] - off[g0]) + i
                            nc.tensor.matmul(
                                psg[:], lhsT=oht[:, ohc * 128:(ohc + 1) * 128],
                                rhs=scall[:, i * GREC:(i + 1) * GREC],
                                start=(i == 0), stop=(i == cg[g] - 1))
                        # normalize + residual + elu -> h2in -> r2 records
                        rec8 = ew.tile([128, H], F32, tag="rec8", name="rec8")
                        nc.vector.reciprocal(rec8[:], psg[:, HID:GREC])
                        t1 = ew.tile([128, HID], F32, tag="t1", name="t1")
                        nc.vector.tensor_tensor(
                            out=t1[:].rearrange("p (h c) -> p h c", h=H),
                            in0=psg[:, 0:HID].rearrange("p (h c) -> p h c", h=H),
                            in1=rec8[:].to_broadcast([128, H, C1_]),
                            op=mybir.AluOpType.mult)
                        nc.vector.tensor_add(t1[:], t1[:], p_sb[:, g * HID:(g + 1) * HID])
                        nc.vector.tensor_add(t1[:], t1[:], BC1T[:])
                        tmin = ew.tile([128, HID], F32, tag="tmin", name="tmin")
                        nc.scalar.activation(tmin[:], t1[:],
                                             mybir.ActivationFunctionType.Relu,
                                             scale=-1.0)
                        texp = ew.tile([128, HID], F32, tag="texp", name="texp")
                        nc.scalar.activation(texp[:], tmin[:],
                                             mybir.ActivationFunctionType.Exp,
                                             scale=-1.0)
                        tmax = ew.tile([128, HID], F32, tag="tmax", name="tmax")
                        nc.scalar.activation(tmax[:], t1[:],
                                             mybir.ActivationFunctionType.Relu)
                        h2sum = ew.tile([128, HID], F32, tag="h2sum", name="h2sum")
                        nc.vector.tensor_add(h2sum[:], texp[:], tmax[:])
                        h2in = ew.tile([128, HID], F32, tag="h2in", name="h2in")
                        nc.vector.tensor_scalar_add(h2in[:], h2sum[:], -1.0)
                        pst = tps.tile([128, HID], F32, tag="pst", name="pst")
                        nc.tensor.transpose(pst[:], h2in[:], ident[:])
                        tT = ew.tile([128, HID], F32, tag="tT", name="tT")
                        nc.vector.tensor_copy(tT[:], pst[:])
                        ps4 = tps.tile([128, 4], F32, tag="ps4", name="ps4")
                        nc.tensor.matmul(ps4[:], lhsT=tT[:], rhs=m4_sb[:],
                                         start=True, stop=True)
                        nc.vector.tensor_copy(r2stage[:, g * 4:(g + 1) * 4], ps4[:])

                # write R2 tables: node-major compact + local a_d2 table
                r2v = r2stage[:].rearrange("p (g r) -> p g r", r=4)
                nc.sync.dma_start(
                    R2loc[:].rearrange("(g p) r -> p g r", p=128), r2v)

            # ---------------- phase D: AllGather R2 + repack
            cc2 = nc.gpsimd.collective_compute(
                "AllGather", mybir.AluOpType.bypass,
                replica_groups=[list(range(NC))],
                ins=[R2loc.opt()], outs=[R2allc.opt()])
            NRP = 8
            rp_chunks = []
            for q in range(NRP):
                q0, q1 = q * (cfg.N // NRP), (q + 1) * (cfg.N // NRP)
                rpq = nc.sync.dma_start(R2main[:][q0:q1, 0:4], R2allc[:][q0:q1, :])
                tile.add_dep_helper(rpq.ins, cc2.ins, sync=True, reason="repack")
                rp_chunks.append(rpq)
            fence_t = pp.tile([1, 1], F32, name="fence_t")
            fence = nc.vector.memset(fence_t[:], 0.0)
            for rpq in rp_chunks:
                tile.add_dep_helper(fence.ins, rpq.ins, sync=True, reason="rpfence")
            rp_insts = [fence]

            R2lo_h = R2main[:][0:HALF, 0:3]
            R2hi_h = R2main[:][HALF:cfg.N, 0:3]

            # ---------------- phase E: layer-2 edge pass
            with (
                tc.tile_pool(name="e2rec", bufs=3) as ep2,
                tc.tile_pool(name="e2work", bufs=4) as ew2,
                tc.tile_pool(name="e2sc", bufs=2) as esc2,
                tc.tile_pool(name="e2oh", bufs=2) as eoh2,
                tc.tile_pool(name="e2psum", bufs=2, space="PSUM") as eps2,
                tc.tile_pool(name="a2psum", bufs=2, space="PSUM") as aps2,
            ):
                for g0 in range(0, G, GB):
                    gs = list(range(g0, min(g0 + GB, G)))
                    nblo = offlo[gs[-1] + 1] - offlo[g0]
                    nbhi = offhi[gs[-1] + 1] - offhi[g0]
                    nb = off[gs[-1] + 1] - off[g0]
                    silo = ep2.tile([128, 8 * nblo], I16, tag="silo2", name="silo2")
                    nc.sync.dma_start(
                        silo[:], SRCLOd[:, 8 * offlo[g0]:8 * (offlo[g0] + nblo)])
                    sihi = ep2.tile([128, 8 * nbhi], I16, tag="sihi2", name="sihi2")
                    nc.sync.dma_start(
                        sihi[:], SRCHId[:, 8 * offhi[g0]:8 * (offhi[g0] + nbhi)])
                    oht2 = eoh2.tile([128, nb * 128], BF16, tag="oht2", name="oht2")
                    nc.sync.dma_start(
                        oht2[:], OHd[:, off[g0] * 128:(off[g0] + nb) * 128])
                    ohtT2 = eoh2.tile([128, nb * 128], BF16, tag="ohtT2", name="ohtT2")
                    nc.sync.dma_start(
                        ohtT2[:], OHTd[:, off[g0] * 128:(off[g0] + nb) * 128])
                    rlo = ep2.tile([128, nblo * 3], F32, tag="rlo", name="rlo")
                    for gi in _gather(nc, rlo[:], 0, R2lo_h, silo[:], 0,
                                      nblo, 3, R2W * 4, nextq()):
                        for _rp in rp_insts:
                            tile.add_dep_helper(gi.ins, _rp.ins, sync=True,
                                                reason="rp1")
                    rhi = ep2.tile([128, nbhi * 3], F32, tag="rhi", name="rhi")
                    for gi in _gather(nc, rhi[:], 0, R2hi_h, sihi[:], 0,
                                      nbhi, 3, R2W * 4, nextq()):
                        for _rp in rp_insts:
                            tile.add_dep_helper(gi.ins, _rp.ins, sync=True,
                                                reason="rp2")
                    for g in gs:
                        gb0 = off[g] - off[g0]
                        ad2g = ew2.tile([128, 1], BF16, tag="ad2g", name="ad2g")
                        nc.vector.tensor_copy(
                            ad2g[:], r2stage[:, g * 4 + 3:g * 4 + 4])
                        adp2 = aps2.tile([128, cg[g]], F32, tag="adp2", name="adp2")
                        for i in range(cg[g]):
                            nc.tensor.matmul(
                                adp2[:, i:i + 1],
                                lhsT=ohtT2[:, (gb0 + i) * 128:(gb0 + i + 1) * 128],
                                rhs=ad2g[:],
                                start=True, stop=True)
                        segs = [
                            (rlo, offlo[g] - offlo[g0], calo[g], 0),
                            (rhi, offhi[g] - offhi[g0], cahi[g], calo[g]),
                        ]
                        sc2 = esc2.tile([128, cg[g] * 3], BF16, tag="sc2", name="sc2")
                        base = off[g] - off[g0]
                        for (rt, lc0, nseg, cbase) in segs:
                            as_ap = bass.AP(
                                rt.tensor, rt[:].offset + lc0 * 3 + 2,
                                [rt[:].ap[0], [3, nseg], [1, 1]])
                            ad_ap = bass.AP(
                                adp2.tensor, adp2[:].offset + cbase,
                                [adp2[:].ap[0], [1, nseg], [1, 1]])
                            epre = ew2.tile([128, nseg], F32, tag="ep2", name="ep2")
                            nc.vector.tensor_tensor(
                                out=epre[:].rearrange("p (n h) -> p n h", h=1),
                                in0=as_ap, in1=ad_ap, op=mybir.AluOpType.add)
                            lr = ew2.tile([128, nseg], F32, tag="lr2", name="lr2")
                            nc.vector.tensor_scalar_mul(lr[:], epre[:], NEG_SLOPE)
                            lrm = ew2.tile([128, nseg], F32, tag="lrm2", name="lrm2")
                            nc.vector.tensor_tensor(
                                out=lrm[:], in0=epre[:], in1=lr[:],
                                op=mybir.AluOpType.max)
                            ex_ap = bass.AP(
                                sc2.tensor, sc2[:].offset + cbase * 3 + 2,
                                [sc2[:].ap[0], [3, nseg], [1, 1]])
                            nc.scalar.activation(
                                ex_ap, lrm[:].rearrange("p (n h) -> p n h", h=1),
                                mybir.ActivationFunctionType.Exp)
                            out4 = bass.AP(
                                sc2.tensor, sc2[:].offset + cbase * 3,
                                [sc2[:].ap[0], [3, nseg], [1, 1], [1, OUT]])
                            in04 = bass.AP(
                                rt.tensor, rt[:].offset + lc0 * 3,
                                [rt[:].ap[0], [3, nseg], [1, 1], [1, OUT]])
                            in14 = bass.AP(
                                sc2.tensor, sc2[:].offset + cbase * 3 + 2,
                                [sc2[:].ap[0], [3, nseg], [1, 1], [0, OUT]])
                            nc.vector.tensor_tensor(
                                out=out4, in0=in04, in1=in14, op=mybir.AluOpType.mult)

                        ps2 = eps2.tile([3, 128], F32, tag="ps2", name="ps2")
                        for i in range(cg[g]):
                            ohc = (off[g] - off[g0]) + i
                            nc.tensor.matmul(
                                ps2[:], lhsT=sc2[:, i * 3:(i + 1) * 3],
                                rhs=oht2[:, ohc * 128:(ohc + 1) * 128],
                                start=(i == 0), stop=(i == cg[g] - 1))
                        nc.vector.tensor_copy(
                            outstage[0:3, g * 128:(g + 1) * 128], ps2[:])
                nc.sync.dma_start(OUTd[:], outstage[:])

    nc.compile()
    return nc


_CACHE = {}


def kernel(x, edge_index, W1, att_src1, att_dst1, b1, Wp, bp,
           W2, att_src2, att_dst2, b2, _trace=False):
    from concourse.bass_utils import run_bass_kernel_spmd
    cfg = _Cfg()
    cfg, in_maps, B2 = _host_prep(
        cfg, x, edge_index, W1, att_src1, att_dst1, b1, Wp, bp,
        W2, att_src2, att_dst2, b2)
    key = (tuple(cfg.calo), tuple(cfg.cahi))
    if key not in _CACHE:
        _CACHE[key] = _build(cfg)
    nc = _CACHE[key]
    res = run_bass_kernel_spmd(
        nc, in_maps, core_ids=list(range(cfg.NC)), trace=_trace)
    out = _unshard(cfg, [res.results[k] for k in range(cfg.NC)], B2)
    kernel.last_exec_time_ns = res.exec_time_ns
    return out


# revision 16
# speedup vs baseline: 1.1824x; 1.1824x over previous
"""nn_GAT — 2-layer PyG-style GAT on 8 TRN2 NeuronCores (Bass/Tile).

Self-contained: kernel(**inputs) takes the FULL unsharded inputs
(as produced by setup_inputs) and returns the FULL [65536, 2] output.

Strategy (graph/data parallel, per sharding hint):
 - nodes sharded 8192/core; edges partitioned by dst core, grouped into
   128-dst-node groups, each group's edges split by src < / >= 32768 (so
   int16 dma_gather indices work via a lo/hi split table) and padded to
   128-edge chunks; all cores share one program (chunk counts maxed).
 - dense phase: per-node record [h(128)|a_s(8)] = x @ [W1 | W1@As] in bf16
   (attention folded in by linearity) + local a_d table in f32 + x@Wp.
 - AllGather the bf16 record table; batched dma_gather (4 SWDGE queues,
   single-packet descriptor streams) pulls 272B records by src and 32B
   a_d rows by (local) dst; segment softmax (shift-free — logits are
   tiny) + message aggregation via one-hot matmuls in PSUM. The one-hot
   tiles are precomputed on the host (static graph) and DMA'd in as bf16,
   shared by both layers.
 - layer 2 repeats with a 16B-row f32 record table [h2|a_s2|a_d2]; its
   segment-sum uses the record as the (3-column) stationary operand so the
   one-hot is the moving operand; final division + bias happen on host
   during unshard (2 flops/output element).
"""
import numpy as np
import ml_dtypes

import concourse.bass as bass
import concourse.bacc as bacc
import concourse.mybir as mybir
import concourse.tile as tile
from concourse.masks import make_identity

F32 = mybir.dt.float32
BF16 = mybir.dt.bfloat16
F8 = mybir.dt.float8e4
I16 = mybir.dt.int16
NEG_SLOPE = 0.2
BF = ml_dtypes.bfloat16
F8NP = ml_dtypes.float8_e4m3

N_NODES = 65536
IN_FEAT = 768
N_CORES = 8
HEADS = 8
C1 = 16
OUT_FEAT = 2
SUP = 512
HALF = 32768  # int16 split point for gather tables
NQ = 4        # SWDGE queues (Q7 core pairs) to spread gathers over


class _Cfg:
    def __init__(self):
        self.N, self.IN, self.NC = N_NODES, IN_FEAT, N_CORES
        self.HEADS, self.C1, self.OUT, self.SUP = HEADS, C1, OUT_FEAT, SUP
        self.calo = []   # per-group lo-segment chunk counts
        self.cahi = []   # per-group hi-segment chunk counts

    @property
    def NL(self):
        return self.N // self.NC

    @property
    def G(self):
        return self.NL // 128

    @property
    def KT(self):
        return self.IN // 128

    @property
    def HID(self):
        return self.HEADS * self.C1

    @property
    def cg(self):
        return [a + b for a, b in zip(self.calo, self.cahi)]

    @property
    def Tlo(self):
        return sum(self.calo)

    @property
    def Thi(self):
        return sum(self.cahi)

    @property
    def T(self):
        return self.Tlo + self.Thi


def _wrap16(vals_by_slot, ncols):
    """vals_by_slot: int array indexed by slot j -> idx value.
    Returns [128, ncols] int16 with idx j at [j%16, j//16], replicated 8x."""
    a = np.zeros((16, ncols), np.int16)
    n = len(vals_by_slot)
    j = np.arange(n)
    a[j % 16, j // 16] = vals_by_slot
    return np.tile(a, (8, 1))


def _host_prep(cfg, x, edge_index, W1, att_src1, att_dst1, b1, Wp, bp,
               W2, att_src2, att_dst2, b2):
    N, NC, NL, G = cfg.N, cfg.NC, cfg.NL, cfg.G
    HID = cfg.HID
    x = np.asarray(x, np.float32)
    ei = np.asarray(edge_index)
    loops = np.arange(N, dtype=np.int64)
    src = np.concatenate([ei[0], loops]).astype(np.int64)
    dst = np.concatenate([ei[1], loops]).astype(np.int64)
    # sort by (dst, src>=HALF) so each 128-dst group splits into lo/hi runs
    order = np.lexsort((src >= HALF, dst // 128))
    src, dst = src[order], dst[order]

    core_of = dst // NL
    core_starts = np.searchsorted(core_of, np.arange(NC + 1))

    # per-core per-group lo/hi counts
    nlo = np.zeros((NC, G), np.int64)
    nhi = np.zeros((NC, G), np.int64)
    for k in range(NC):
        s, e = core_starts[k], core_starts[k + 1]
        g = (dst[s:e] - k * NL) // 128
        hi = (src[s:e] >= HALF).astype(np.int64)
        nlo[k] = np.bincount(g, weights=1 - hi, minlength=G)
        nhi[k] = np.bincount(g, weights=hi, minlength=G)
    calo = np.maximum((-(-nlo.astype(np.int64) // 128)).max(axis=0), 1)
    cahi = np.maximum((-(-nhi.astype(np.int64) // 128)).max(axis=0), 1)
    cfg.calo = [int(c) for c in calo]
    cfg.cahi = [int(c) for c in cahi]
    cg = calo + cahi
    offlo = np.concatenate([[0], np.cumsum(calo)])
    offhi = np.concatenate([[0], np.cumsum(cahi)])
    off = np.concatenate([[0], np.cumsum(cg)])
    Tlo, Thi, T = int(calo.sum()), int(cahi.sum()), int(cg.sum())

    W1 = np.asarray(W1, np.float32)
    Wp = np.asarray(Wp, np.float32)
    As = np.zeros((HID, cfg.HEADS), np.float32)
    Ad = np.zeros((HID, cfg.HEADS), np.float32)
    hh = np.repeat(np.arange(cfg.HEADS), cfg.C1)
    As[np.arange(HID), hh] = np.asarray(att_src1, np.float32).ravel()
    Ad[np.arange(HID), hh] = np.asarray(att_dst1, np.float32).ravel()
    WBIG = np.concatenate([W1, W1 @ As, W1 @ Ad, Wp], axis=1)  # [IN, 272]

    W2 = np.asarray(W2, np.float32)
    M4 = np.concatenate(
        [W2, W2 @ np.asarray(att_src2, np.float32).T,
         W2 @ np.asarray(att_dst2, np.float32).T], axis=1)     # [HID, 4]
    BC1 = (np.asarray(b1, np.float32) + np.asarray(bp, np.float32))[None, :]
    B2 = np.asarray(b2, np.float32)

    in_maps = []
    for k in range(NC):
        s, e = core_starts[k], core_starts[k + 1]
        sk, dk = src[s:e], dst[s:e]
        gk = (dk - k * NL) // 128
        hik = sk >= HALF
        # rank within (group, seg): edges are sorted by (group, hi) so
        # positions within each (g, seg) run are consecutive
        pos = np.arange(e - s)
        seg_key = gk * 2 + hik
        seg_start = np.concatenate([[0], np.cumsum(np.bincount(
            seg_key, minlength=2 * G))])
        r_in_seg = pos - seg_start[seg_key]
        # chunk slot j within the group's chunk space
        j_lo = r_in_seg                     # for lo edges
        j_hi = calo[gk] * 128 + r_in_seg    # for hi edges
        j = np.where(hik, j_hi, j_lo)
        chunk = off[gk] + j // 128          # global chunk index
        lane = j % 128

        # lo/hi gather slot (position within that segment's idx stream)
        slot_lo = (offlo[gk] * 128 + r_in_seg)[~hik]
        slot_hi = (offhi[gk] * 128 + r_in_seg)[hik]
        vals = np.zeros(Tlo * 128, np.int64)
        vals[slot_lo] = sk[~hik]
        SRCLO = _wrap16(vals, 8 * Tlo)
        vals = np.zeros(Thi * 128, np.int64)
        vals[slot_hi] = sk[hik] - HALF
        SRCHI = _wrap16(vals, 8 * Thi)
        # one-hot tiles (static graph): chunk c col-block holds
        # oh[lane, dstpos] = 1 for each real edge; padding rows all-zero,
        # plus the transposed tiles for dst->edge broadcasts (a_d terms)
        dpos = (dk - k * NL) % 128
        OH = np.zeros((128, T * 128), F8NP)
        OH[lane, chunk * 128 + dpos] = 1
        OHT = np.zeros((128, T * 128), F8NP)
        OHT[dpos, chunk * 128 + lane] = 1

        in_maps.append({
            "XT": np.ascontiguousarray(
                x[k * NL:(k + 1) * NL].T).astype(BF),
            "WBIG": WBIG.astype(BF), "M4": M4, "BC1": BC1,
            "SRCLO": SRCLO, "SRCHI": SRCHI, "OH": OH, "OHT": OHT,
        })
    return cfg, in_maps, B2


def _unshard(cfg, outs, B2):
    parts = []
    for k in range(cfg.NC):
        o = outs[k]["OUT"]                       # [4, G*128]
        num = o[0:2].reshape(2, cfg.G, 128)
        den = o[2].reshape(cfg.G, 128)
        r = (num / den[None]).transpose(1, 2, 0).reshape(cfg.NL, 2)
        parts.append(r + B2[None, :])
    return np.concatenate(parts, axis=0).astype(np.float32)


MAX_GCH = 16  # chunks (x128 idxs) per dma_gather call


def _gather(nc, out_tile, out_col0, in_ap, idx_tile, idx_col0, nchunks, elem,
            stride_bytes, nextq):
    """Batched dma_gather, split into <=MAX_GCH-chunk calls, each call on
    the next SWDGE queue (Q7 core pair) round-robin.
    out rows: [128, nchunks, elem] at out_tile cols out_col0*elem;
    idx cols: idx_tile[:, 8*idx_col0 : 8*(idx_col0+nchunks)]."""
    eng = nc.gpsimd
    insts = []
    for c0 in range(0, nchunks, MAX_GCH):
        queue_num = nextq()
        nch = min(MAX_GCH, nchunks - c0)
        o = out_tile[:, (out_col0 + c0) * elem:(out_col0 + c0 + nch) * elem]
        o = o.rearrange("p (n e) -> p n e", e=elem)
        ix = idx_tile[:, 8 * (idx_col0 + c0):8 * (idx_col0 + c0 + nch)]
        insts.append(eng.add_instruction(
            mybir.InstDMAGatherAnt(
                name=nc.get_next_instruction_name(),
                ins=[*eng.lower_ap_dma(in_ap, for_custom_bir_dma=True),
                     eng.lower_ap(ix),
                     eng.lower_val_access(eng.to_reg(nch * 128))],
                outs=[eng.lower_ap(o)],
                transpose=False,
                num_idxs=nch * 128,
                elem_size=elem,
                stride_bytes_256=stride_bytes // 256,
                gen_mode=0,
                single_packet=False,
                queue_num=queue_num,
            )))
    return insts


def _build(cfg):
    NC = cfg.NC
    NL, G, KT = cfg.NL, cfg.G, cfg.KT
    H, C1_, HID, OUT = cfg.HEADS, cfg.C1, cfg.HID, cfg.OUT
    REC = HID + 2 * H        # 144 (gemm out: h|a_s|a_d)
    GREC = HID + H           # 136 (gathered: h|a_s)
    TROW = 256               # bf16 elements per T1main row (512B)
    ADW = 64                 # f32 per T1ad row (256B)
    R2W = 64                 # f32 per R2main row (256B)
    SUP_ = cfg.SUP
    calo, cahi, cg = cfg.calo, cfg.cahi, cfg.cg
    Tlo, Thi, T = cfg.Tlo, cfg.Thi, cfg.T
    offlo = [0]
    for c in calo:
        offlo.append(offlo[-1] + c)
    offhi = [0]
    for c in cahi:
        offhi.append(offhi[-1] + c)
    off = [0]
    for c in cg:
        off.append(off[-1] + c)

    qrot = [0]

    def nextq():
        q = qrot[0]
        qrot[0] = (q + 1) % NQ
        return q

    nc = bacc.Bacc("TRN2", target_bir_lowering=False, debug=False,
                   num_devices=NC, num_swdge_queues=NQ)
    XT = nc.dram_tensor("XT", [cfg.IN, NL], BF16, kind="ExternalInput")
    WBIGd = nc.dram_tensor("WBIG", [cfg.IN, REC + HID], BF16, kind="ExternalInput")
    M4d = nc.dram_tensor("M4", [HID, 4], F32, kind="ExternalInput")
    BC1d = nc.dram_tensor("BC1", [1, HID], F32, kind="ExternalInput")
    SRCLOd = nc.dram_tensor("SRCLO", [128, 8 * Tlo], I16, kind="ExternalInput")
    SRCHId = nc.dram_tensor("SRCHI", [128, 8 * Thi], I16, kind="ExternalInput")
    OHd = nc.dram_tensor("OH", [128, T * 128], F8, kind="ExternalInput")
    OHTd = nc.dram_tensor("OHT", [128, T * 128], F8, kind="ExternalInput")
    OUTd = nc.dram_tensor("OUT", [4, G * 128], F32, kind="ExternalOutput")

    with tile.TileContext(nc) as tc:
        with (
            tc.tile_pool(name="dram", bufs=1, space="DRAM") as dram,
            tc.tile_pool(name="const", bufs=1) as cb,
            tc.tile_pool(name="persist", bufs=1) as pp,
        ):
            T1loc = dram.tile([NL, TROW], BF16)
            T1main = dram.tile([cfg.N, TROW], BF16)
            R2loc = dram.tile([NL, 4], F32)
            R2allc = dram.tile([cfg.N, 4], F32)
            R2main = dram.tile([cfg.N, R2W], F32)

            ident = cb.tile([128, 128], F32)
            make_identity(nc, ident[:])
            wb_sb = []
            for kk in range(KT):
                t = cb.tile([128, REC + HID], BF16, tag=f"wb{kk}", name=f"wb{kk}")
                nc.sync.dma_start(t[:], WBIGd[kk * 128:(kk + 1) * 128, :])
                wb_sb.append(t)
            m4_sb = cb.tile([HID, 4], F32)
            nc.sync.dma_start(m4_sb[:], M4d[:])
            ones1 = cb.tile([1, 128], F32)
            nc.vector.memset(ones1[:], 1.0)
            bc1row = cb.tile([1, HID], F32)
            nc.sync.dma_start(bc1row[:], BC1d[:])
            with tc.tile_pool(name="bpsum", bufs=1, space="PSUM") as bps:
                bp1 = bps.tile([128, HID], F32)
                nc.tensor.matmul(bp1[:], lhsT=ones1[:], rhs=bc1row[:], start=True, stop=True)
                BC1T = cb.tile([128, HID], F32)
                nc.vector.tensor_copy(BC1T[:], bp1[:])

            p_sb = pp.tile([128, G * HID], F32)
            ad_sb = pp.tile([128, G * H], BF16)
            r2stage = pp.tile([128, G * 4], F32)
            outstage = pp.tile([4, G * 128], F32)
            nc.vector.memset(outstage[:], 0.0)

            # ---------------- phase A: GEMM
            n_sup = NL // SUP_
            m_per = SUP_ // 128
            with (
                tc.tile_pool(name="xts", bufs=2 * KT) as xp,
                tc.tile_pool(name="gpsum", bufs=3, space="PSUM") as gps,
                tc.tile_pool(name="grec", bufs=3) as grp,
            ):
                for s in range(n_sup):
                    xts = []
                    for kk in range(KT):
                        t = xp.tile([128, SUP_], BF16, tag="xts", name="xts")
                        nc.sync.dma_start(
                            t[:], XT[kk * 128:(kk + 1) * 128, s * SUP_:(s + 1) * SUP_])
                        xts.append(t)
                    for m in range(m_per):
                        gm = s * m_per + m
                        ps = gps.tile([128, REC + HID], F32, tag="gp", name="gp")
                        for kk in range(KT):
                            nc.tensor.matmul(
                                ps[:], lhsT=xts[kk][:, m * 128:(m + 1) * 128],
                                rhs=wb_sb[kk][:], start=(kk == 0), stop=(kk == KT - 1))
                        rec = grp.tile([128, TROW], BF16, tag="rec", name="rec")
                        nc.vector.tensor_copy(rec[:, 0:GREC], ps[:, 0:GREC])
                        nc.vector.tensor_copy(
                            ad_sb[:, gm * H:(gm + 1) * H], ps[:, GREC:REC])
                        nc.vector.tensor_copy(
                            p_sb[:, gm * HID:(gm + 1) * HID], ps[:, REC:REC + HID])
                        nc.sync.dma_start(T1loc[gm * 128:(gm + 1) * 128, :], rec[:])

            # ---------------- phase B: AllGather T1 (padded bf16 rows)
            cc1 = nc.gpsimd.collective_compute(
                "AllGather", mybir.AluOpType.bypass,
                replica_groups=[list(range(NC))],
                ins=[T1loc.opt()], outs=[T1main.opt()])

            T1lo_h = T1main[:][0:HALF, 0:GREC]
            T1hi_h = T1main[:][HALF:cfg.N, 0:GREC]

            # ---------------- phase C: layer-1 edge pass + layer-2 prep
            GB = 2  # groups per gather batch
            with (
                tc.tile_pool(name="erec", bufs=2) as ep,
                tc.tile_pool(name="ework", bufs=4) as ew,
                tc.tile_pool(name="escall", bufs=2) as esc,
                tc.tile_pool(name="eoh", bufs=2) as eoh,
                tc.tile_pool(name="epsum", bufs=2, space="PSUM") as eps,
                tc.tile_pool(name="apsum", bufs=2, space="PSUM") as aps,
                tc.tile_pool(name="tpsum", bufs=2, space="PSUM") as tps,
            ):
                for g0 in range(0, G, GB):
                    gs = list(range(g0, min(g0 + GB, G)))
                    nblo = offlo[gs[-1] + 1] - offlo[g0]
                    nbhi = offhi[gs[-1] + 1] - offhi[g0]
                    nb = off[gs[-1] + 1] - off[g0]
                    silo = ep.tile([128, 8 * nblo], I16, tag="silo", name="silo")
                    nc.sync.dma_start(
                        silo[:], SRCLOd[:, 8 * offlo[g0]:8 * (offlo[g0] + nblo)])
                    sihi = ep.tile([128, 8 * nbhi], I16, tag="sihi", name="sihi")
                    nc.sync.dma_start(
                        sihi[:], SRCHId[:, 8 * offhi[g0]:8 * (offhi[g0] + nbhi)])
                    oht = eoh.tile([128, nb * 128], F8, tag="oht", name="oht")
                    nc.sync.dma_start(
                        oht[:], OHd[:, off[g0] * 128:(off[g0] + nb) * 128])
                    ohtT = eoh.tile([128, nb * 128], F8, tag="ohtT", name="ohtT")
                    nc.sync.dma_start(
                        ohtT[:], OHTd[:, off[g0] * 128:(off[g0] + nb) * 128])
                    hlo = ep.tile([128, nblo * GREC], BF16, tag="hlo", name="hlo")
                    for gi in _gather(nc, hlo[:], 0, T1lo_h, silo[:], 0,
                                      nblo, GREC, TROW * 2, nextq):
                        tile.add_dep_helper(gi.ins, cc1.ins, sync=True, reason="ag1")
                    hhi = ep.tile([128, nbhi * GREC], BF16, tag="hhi", name="hhi")
                    for gi in _gather(nc, hhi[:], 0, T1hi_h, sihi[:], 0,
                                      nbhi, GREC, TROW * 2, nextq):
                        tile.add_dep_helper(gi.ins, cc1.ins, sync=True, reason="ag1")
                    for g in gs:
                        # per-edge a_d via one-hot-transpose matmuls (PSUM)
                        gb0 = off[g] - off[g0]
                        adp = aps.tile([128, cg[g] * H], F32, tag="adp", name="adp")
                        for i in range(cg[g]):
                            nc.tensor.matmul(
                                adp[:, i * H:(i + 1) * H],
                                lhsT=ohtT[:, (gb0 + i) * 128:(gb0 + i + 1) * 128],
                                rhs=ad_sb[:, g * H:(g + 1) * H],
                                start=True, stop=True)
                        # segment (tile, local chunk range) resolution
                        segs = [
                            (hlo, offlo[g] - offlo[g0], calo[g], 0),
                            (hhi, offhi[g] - offhi[g0], cahi[g], calo[g]),
                        ]
                        scall = esc.tile([128, cg[g] * GREC], BF16,
                                         tag="scall", name="scall")
                        base = off[g] - off[g0]
                        for (ht, lc0, nseg, cbase) in segs:
                            # group+segment-wide batched ops
                            as_ap = bass.AP(
                                ht.tensor, ht[:].offset + lc0 * GREC + HID,
                                [ht[:].ap[0], [GREC, nseg], [1, H]])
                            ad_ap = bass.AP(
                                adp.tensor,
                                adp[:].offset + cbase * H,
                                [adp[:].ap[0], [H, nseg], [1, H]])
                            epre = ew.tile([128, nseg * H], F32, tag="epre", name="epre")
                            nc.vector.tensor_tensor(
                                out=epre[:].rearrange("p (n h) -> p n h", h=H),
                                in0=as_ap, in1=ad_ap, op=mybir.AluOpType.add)
                            lr = ew.tile([128, nseg * H], F32, tag="lr", name="lr")
                            nc.vector.tensor_scalar_mul(lr[:], epre[:], NEG_SLOPE)
                            lrm = ew.tile([128, nseg * H], F32, tag="lrm", name="lrm")
                            nc.vector.tensor_tensor(
                                out=lrm[:], in0=epre[:], in1=lr[:],
                                op=mybir.AluOpType.max)
                            ex_ap = bass.AP(
                                scall.tensor, scall[:].offset + cbase * GREC + HID,
                                [scall[:].ap[0], [GREC, nseg], [1, H]])
                            nc.scalar.activation(
                                ex_ap, lrm[:].rearrange("p (n h) -> p n h", h=H),
                                mybir.ActivationFunctionType.Exp)
                            # 4-dim scaled-message mul (bf16 h x bf16 ex -> bf16)
                            out4 = bass.AP(
                                scall.tensor, scall[:].offset + cbase * GREC,
                                [scall[:].ap[0], [GREC, nseg], [C1_, H], [1, C1_]])
                            in04 = bass.AP(
                                ht.tensor, ht[:].offset + lc0 * GREC,
                                [ht[:].ap[0], [GREC, nseg], [C1_, H], [1, C1_]])
                            in14 = bass.AP(
                                scall.tensor, scall[:].offset + cbase * GREC + HID,
                                [scall[:].ap[0], [GREC, nseg], [1, H], [0, C1_]])
                            nc.vector.tensor_tensor(
                                out=out4, in0=in04, in1=in14, op=mybir.AluOpType.mult)

                        psg = eps.tile([128, GREC], F32, tag="psg", name="psg")
                        for i in range(cg[g]):
                            ohc = (off[g] - off[g0]) + i
                            nc.tensor.matmul(
                                psg[:], lhsT=oht[:, ohc * 128:(ohc + 1) * 128],
                                rhs=scall[:, i * GREC:(i + 1) * GREC],
                                start=(i == 0), stop=(i == cg[g] - 1))
                        # normalize + residual + elu -> h2in -> r2 records
                        rec8 = ew.tile([128, H], F32, tag="rec8", name="rec8")
                        nc.vector.reciprocal(rec8[:], psg[:, HID:GREC])
                        t1 = ew.tile([128, HID], F32, tag="t1", name="t1")
                        nc.vector.tensor_tensor(
                            out=t1[:].rearrange("p (h c) -> p h c", h=H),
                            in0=psg[:, 0:HID].rearrange("p (h c) -> p h c", h=H),
                            in1=rec8[:].to_broadcast([128, H, C1_]),
                            op=mybir.AluOpType.mult)
                        nc.vector.tensor_add(t1[:], t1[:], p_sb[:, g * HID:(g + 1) * HID])
                        nc.vector.tensor_add(t1[:], t1[:], BC1T[:])
                        tmin = ew.tile([128, HID], F32, tag="tmin", name="tmin")
                        nc.scalar.activation(tmin[:], t1[:],
                                             mybir.ActivationFunctionType.Relu,
                                             scale=-1.0)
                        texp = ew.tile([128, HID], F32, tag="texp", name="texp")
                        nc.scalar.activation(texp[:], tmin[:],
                                             mybir.ActivationFunctionType.Exp,
                                             scale=-1.0)
                        tmax = ew.tile([128, HID], F32, tag="tmax", name="tmax")
                        nc.scalar.activation(tmax[:], t1[:],
                                             mybir.ActivationFunctionType.Relu)
                        h2sum = ew.tile([128, HID], F32, tag="h2sum", name="h2sum")
                        nc.vector.tensor_add(h2sum[:], texp[:], tmax[:])
                        h2in = ew.tile([128, HID], F32, tag="h2in", name="h2in")
                        nc.vector.tensor_scalar_add(h2in[:], h2sum[:], -1.0)
                        pst = tps.tile([128, HID], F32, tag="pst", name="pst")
                        nc.tensor.transpose(pst[:], h2in[:], ident[:])
                        tT = ew.tile([128, HID], F32, tag="tT", name="tT")
                        nc.vector.tensor_copy(tT[:], pst[:])
                        ps4 = tps.tile([128, 4], F32, tag="ps4", name="ps4")
                        nc.tensor.matmul(ps4[:], lhsT=tT[:], rhs=m4_sb[:],
                                         start=True, stop=True)
                        nc.vector.tensor_copy(r2stage[:, g * 4:(g + 1) * 4], ps4[:])

                # write R2 tables: node-major compact + local a_d2 table
                r2v = r2stage[:].rearrange("p (g r) -> p g r", r=4)
                nc.sync.dma_start(
                    R2loc[:].rearrange("(g p) r -> p g r", p=128), r2v)

            # ---------------- phase D: AllGather R2 + repack
            cc2 = nc.gpsimd.collective_compute(
                "AllGather", mybir.AluOpType.bypass,
                replica_groups=[list(range(NC))],
                ins=[R2loc.opt()], outs=[R2allc.opt()])
            NRP = 8
            rp_chunks = []
            for q in range(NRP):
                q0, q1 = q * (cfg.N // NRP), (q + 1) * (cfg.N // NRP)
                rpq = nc.sync.dma_start(R2main[:][q0:q1, 0:4], R2allc[:][q0:q1, :])
                tile.add_dep_helper(rpq.ins, cc2.ins, sync=True, reason="repack")
                rp_chunks.append(rpq)
            fence_t = pp.tile([1, 1], F32, name="fence_t")
            fence = nc.vector.memset(fence_t[:], 0.0)
            for rpq in rp_chunks:
                tile.add_dep_helper(fence.ins, rpq.ins, sync=True, reason="rpfence")
            rp_insts = [fence]

            R2lo_h = R2main[:][0:HALF, 0:3]
            R2hi_h = R2main[:][HALF:cfg.N, 0:3]

            # ---------------- phase E: layer-2 edge pass
            with (
                tc.tile_pool(name="e2rec", bufs=2) as ep2,
                tc.tile_pool(name="e2work", bufs=4) as ew2,
                tc.tile_pool(name="e2sc", bufs=2) as esc2,
                tc.tile_pool(name="e2oh", bufs=2) as eoh2,
                tc.tile_pool(name="e2psum", bufs=2, space="PSUM") as eps2,
                tc.tile_pool(name="a2psum", bufs=2, space="PSUM") as aps2,
            ):
                for g0 in range(0, G, GB):
                    gs = list(range(g0, min(g0 + GB, G)))
                    nblo = offlo[gs[-1] + 1] - offlo[g0]
                    nbhi = offhi[gs[-1] + 1] - offhi[g0]
                    nb = off[gs[-1] + 1] - off[g0]
                    silo = ep2.tile([128, 8 * nblo], I16, tag="silo2", name="silo2")
                    nc.sync.dma_start(
                        silo[:], SRCLOd[:, 8 * offlo[g0]:8 * (offlo[g0] + nblo)])
                    sihi = ep2.tile([128, 8 * nbhi], I16, tag="sihi2", name="sihi2")
                    nc.sync.dma_start(
                        sihi[:], SRCHId[:, 8 * offhi[g0]:8 * (offhi[g0] + nbhi)])
                    oht2 = eoh2.tile([128, nb * 128], F8, tag="oht2", name="oht2")
                    nc.sync.dma_start(
                        oht2[:], OHd[:, off[g0] * 128:(off[g0] + nb) * 128])
                    ohtT2 = eoh2.tile([128, nb * 128], F8, tag="ohtT2", name="ohtT2")
                    nc.sync.dma_start(
                        ohtT2[:], OHTd[:, off[g0] * 128:(off[g0] + nb) * 128])
                    rlo = ep2.tile([128, nblo * 3], F32, tag="rlo", name="rlo")
                    for gi in _gather(nc, rlo[:], 0, R2lo_h, silo[:], 0,
                                      nblo, 3, R2W * 4, nextq):
                        for _rp in rp_insts:
                            tile.add_dep_helper(gi.ins, _rp.ins, sync=True,
                                                reason="rp1")
                    rhi = ep2.tile([128, nbhi * 3], F32, tag="rhi", name="rhi")
                    for gi in _gather(nc, rhi[:], 0, R2hi_h, sihi[:], 0,
                                      nbhi, 3, R2W * 4, nextq):
                        for _rp in rp_insts:
                            tile.add_dep_helper(gi.ins, _rp.ins, sync=True,
                                                reason="rp2")
                    for g in gs:
                        gb0 = off[g] - off[g0]
                        ad2g = ew2.tile([128, 1], BF16, tag="ad2g", name="ad2g")
                        nc.vector.tensor_copy(
                            ad2g[:], r2stage[:, g * 4 + 3:g * 4 + 4])
                        adp2 = aps2.tile([128, cg[g]], F32, tag="adp2", name="adp2")
                        for i in range(cg[g]):
                            nc.tensor.matmul(
                                adp2[:, i:i + 1],
                                lhsT=ohtT2[:, (gb0 + i) * 128:(gb0 + i + 1) * 128],
                                rhs=ad2g[:],
                                start=True, stop=True)
                        segs = [
                            (rlo, offlo[g] - offlo[g0], calo[g], 0),
                            (rhi, offhi[g] - offhi[g0], cahi[g], calo[g]),
                        ]
                        sc2 = esc2.tile([128, cg[g] * 3], BF16, tag="sc2", name="sc2")
                        base = off[g] - off[g0]
                        for (rt, lc0, nseg, cbase) in segs:
                            as_ap = bass.AP(
                                rt.tensor, rt[:].offset + lc0 * 3 + 2,
                                [rt[:].ap[0], [3, nseg], [1, 1]])
                            ad_ap = bass.AP(
                                adp2.tensor, adp2[:].offset + cbase,
                                [adp2[:].ap[0], [1, nseg], [1, 1]])
                            epre = ew2.tile([128, nseg], F32, tag="ep2", name="ep2")
                            nc.vector.tensor_tensor(
                                out=epre[:].rearrange("p (n h) -> p n h", h=1),
                                in0=as_ap, in1=ad_ap, op=mybir.AluOpType.add)
                            lr = ew2.tile([128, nseg], F32, tag="lr2", name="lr2")
                            nc.vector.tensor_scalar_mul(lr[:], epre[:], NEG_SLOPE)
                            lrm = ew2.tile([128, nseg], F32, tag="lrm2", name="lrm2")
                            nc.vector.tensor_tensor(
                                out=lrm[:], in0=epre[:], in1=lr[:],
                                op=mybir.AluOpType.max)
                            ex_ap = bass.AP(
                                sc2.tensor, sc2[:].offset + cbase * 3 + 2,
                                [sc2[:].ap[0], [3, nseg], [1, 1]])
                            nc.scalar.activation(
                                ex_ap, lrm[:].rearrange("p (n h) -> p n h", h=1),
                                mybir.ActivationFunctionType.Exp)
                            out4 = bass.AP(
                                sc2.tensor, sc2[:].offset + cbase * 3,
                                [sc2[:].ap[0], [3, nseg], [1, 1], [1, OUT]])
                            in04 = bass.AP(
                                rt.tensor, rt[:].offset + lc0 * 3,
                                [rt[:].ap[0], [3, nseg], [1, 1], [1, OUT]])
                            in14 = bass.AP(
                                sc2.tensor, sc2[:].offset + cbase * 3 + 2,
                                [sc2[:].ap[0], [3, nseg], [1, 1], [0, OUT]])
                            nc.vector.tensor_tensor(
                                out=out4, in0=in04, in1=in14, op=mybir.AluOpType.mult)

                        ps2 = eps2.tile([3, 128], F32, tag="ps2", name="ps2")
                        for i in range(cg[g]):
                            ohc = (off[g] - off[g0]) + i
                            nc.tensor.matmul(
                                ps2[:], lhsT=sc2[:, i * 3:(i + 1) * 3],
                                rhs=oht2[:, ohc * 128:(ohc + 1) * 128],
                                start=(i == 0), stop=(i == cg[g] - 1))
                        nc.vector.tensor_copy(
                            outstage[0:3, g * 128:(g + 1) * 128], ps2[:])
                nc.sync.dma_start(OUTd[:], outstage[:])

    nc.compile()
    return nc


_CACHE = {}


def kernel(x, edge_index, W1, att_src1, att_dst1, b1, Wp, bp,
           W2, att_src2, att_dst2, b2, _trace=False):
    from concourse.bass_utils import run_bass_kernel_spmd
    cfg = _Cfg()
    cfg, in_maps, B2 = _host_prep(
        cfg, x, edge_index, W1, att_src1, att_dst1, b1, Wp, bp,
        W2, att_src2, att_dst2, b2)
    key = (tuple(cfg.calo), tuple(cfg.cahi))
    if key not in _CACHE:
        _CACHE[key] = _build(cfg)
    nc = _CACHE[key]
    res = run_bass_kernel_spmd(
        nc, in_maps, core_ids=list(range(cfg.NC)), trace=_trace)
    out = _unshard(cfg, [res.results[k] for k in range(cfg.NC)], B2)
    kernel.last_exec_time_ns = res.exec_time_ns
    return out
